# revision 7
# baseline (speedup 1.0000x reference)
"""Trainium2 Bass kernel for a Keras-style GRU encoder (reset_after=True).

Math (per core, batch shard bb=16 of B=128):
  x_proj = x @ W + b_i          (precomputed for all T in a streaming GEMM)
  per t: rec = h @ U + b_r
         z = sigmoid(xp_z + rec_z); r = sigmoid(xp_r + rec_r)
         hh = tanh(xp_h + r * rec_h)
         h  = z*h + (1-z)*hh

Layouts: the scan runs feature-major ("transposed"): h^T is [128 part, 2, 16]
(feature chunk k on partitions, batch on free), rec^T accumulates in PSUM as
[128, 6, 16] via 12 matmuls with U chunks as the stationary operand. Gate
elementwise ops are [128, 2|4, 16] tiles (all 128 DVE/ACT lanes active).
x_proj is produced feature-major into a DRAM scratch ([6, 128, T*16],
col = t*16 + batch) by a streaming GEMM that PE-transposes x tiles; the
combined bias (b_i + b_r for z/r, b_i for the h gate) is folded in there.
Outputs transpose back to batch-major via PE every 8 steps.
"""
import sys

sys.path.insert(0, "/opt/trn_rl_repo")

import numpy as np
import concourse.bass as bass
import concourse.bacc as bacc
import concourse.tile as tile
from concourse import mybir
from concourse.bass_utils import run_bass_kernel_spmd
from concourse.masks import make_identity

B, T, IN, H = 128, 1024, 128, 256
G3 = 3 * H          # 768 = 6 chunks of 128
NCORES = 8
BB = B // NCORES    # 16 batch per core
NB = 8              # timesteps per output/fetch block
NG = T // NB
F32 = mybir.dt.float32

_cache = {}


def _build(t_total=T):
    ng = t_total // NB
    nc = bacc.Bacc("TRN2", target_bir_lowering=False, debug=False,
                   num_devices=NCORES)
    x_h = nc.declare_dram_parameter("x", [BB, t_total, IN], F32, isOutput=False)
    hid_h = nc.declare_dram_parameter("hidden", [BB, H], F32, isOutput=False)
    w_h = nc.declare_dram_parameter("W", [IN, G3], F32, isOutput=False)
    u_h = nc.declare_dram_parameter("U", [H, G3], F32, isOutput=False)
    b_h = nc.declare_dram_parameter("b", [2, G3], F32, isOutput=False)
    out_h = nc.declare_dram_parameter("out", [BB, t_total, H], F32, isOutput=True)
    st_h = nc.declare_dram_parameter("state", [BB, H], F32, isOutput=True)

    with tile.TileContext(nc) as tc:
        with (
            tc.tile_pool(name="consts", bufs=1) as consts,
            tc.tile_pool(name="xin", bufs=3) as xin,
            tc.tile_pool(name="xts", bufs=3) as xts,
            tc.tile_pool(name="xpsb", bufs=3) as xpsb,
            tc.tile_pool(name="xp8", bufs=3) as xp8p,
            tc.tile_pool(name="hist", bufs=3) as histp,
            tc.tile_pool(name="gates", bufs=3) as gates,
            tc.tile_pool(name="gemm_ps", bufs=2, space="PSUM") as gemm_ps,
            tc.tile_pool(name="scan_ps", bufs=2, space="PSUM") as scan_ps,
            tc.tile_pool(name="out_ps", bufs=2, space="PSUM") as out_ps,
            tc.tile_pool(name="dram", bufs=1, space="DRAM") as dram,
        ):
            # ---- constants -------------------------------------------------
            identity = consts.tile([128, 128], F32)
            make_identity(nc, identity)
            u0 = consts.tile([128, G3], F32)
            u1 = consts.tile([128, G3], F32)
            nc.sync.dma_start(out=u0, in_=u_h[0:128, :])
            nc.sync.dma_start(out=u1, in_=u_h[128:256, :])
            w_sb = consts.tile([128, G3], F32)
            nc.sync.dma_start(out=w_sb, in_=w_h[:])
            # b [2, 768] -> [128 part(p), 2, 6(m)]
            b_sb = consts.tile([128, 2, 6], F32)
            nc.sync.dma_start(out=b_sb,
                              in_=b_h[:].rearrange("two (m p) -> p two m", p=128))
            # per-(partition, chunk) bias column folded into x_proj:
            # z/r chunks get b_i + b_r; h chunks get b_i only.
            bias_cols = consts.tile([128, 6], F32)
            nc.vector.tensor_add(bias_cols, b_sb[:, 0, :], b_sb[:, 1, :])
            nc.vector.tensor_copy(bias_cols[:, 4:6], b_sb[:, 0, 4:6])
            # recurrent bias for the h gate, broadcast over batch: [128, 2, 16]
            zeros16 = consts.tile([128, 16], F32)
            nc.vector.memset(zeros16, 0.0)
            bh_rh = consts.tile([128, 2, 16], F32)
            for c in range(2):
                nc.vector.tensor_scalar_add(bh_rh[:, c, :], zeros16,
                                            b_sb[:, 1, 4 + c:5 + c])

            # ---- initial hidden -> hT_init [128, 2, 16] --------------------
            hid_sb = consts.tile([BB, H], F32)
            nc.sync.dma_start(out=hid_sb, in_=hid_h[:])
            hid_ps = out_ps.tile([128, 2, 16], F32, tag="ops")
            for c in range(2):
                nc.tensor.transpose(hid_ps[:, c, :], hid_sb[:, c * 128:(c + 1) * 128],
                                    identity[0:BB, 0:BB])
            ht_init = consts.tile([128, 2, 16], F32)
            nc.vector.tensor_copy(ht_init, hid_ps)

            # x_proj^T scratch: [6(m), 128(p), t*16 + j]
            xpt_d = dram.tile([6, 128, t_total * BB], F32)

            prev = ht_init  # [128, 2, 16] view of h_{t-1}

            for g in range(ng):
                # ---- GEMM block: x_proj^T for timesteps [g*8, g*8+8) -------
                x_tile = xin.tile([128, IN], F32)
                # src iteration order (t, b, i): composite (t,b) partition dim
                x_ap = x_h[:]
                src = bass.AP(tensor=x_ap.tensor, offset=x_ap.offset + g * NB * IN,
                              ap=[[IN, NB], [t_total * IN, BB], [1, IN]])
                nc.sync.dma_start(out=x_tile, in_=src)
                xt_ps = out_ps.tile([128, 128], F32, tag="ops")
                nc.tensor.transpose(xt_ps, x_tile, identity)
                xt_sb = xts.tile([128, 128], F32)
                nc.any.tensor_copy(xt_sb, xt_ps)
                psum_g = gemm_ps.tile([128, 6, 128], F32)
                for m in range(6):
                    nc.tensor.matmul(psum_g[:, m, :], w_sb[:, m * 128:(m + 1) * 128],
                                     xt_sb, start=True, stop=True)
                xp_sb = xpsb.tile([128, 6, 128], F32)
                for m in range(6):
                    nc.any.tensor_scalar_add(xp_sb[:, m, :], psum_g[:, m, :],
                                             bias_cols[:, m:m + 1])
                nc.sync.dma_start(
                    out=xpt_d[:, :, g * 128:(g + 1) * 128].rearrange("m p c -> p m c"),
                    in_=xp_sb)

                # ---- scan: 8 sequential GRU steps --------------------------
                xp8 = xp8p.tile([128, 6, NB * BB], F32)
                nc.sync.dma_start(
                    out=xp8,
                    in_=xpt_d[:, :, g * 128:(g + 1) * 128].rearrange("m p c -> p m c"))
                hist = histp.tile([128, 2, NB, 16], F32)
                for s in range(NB):
                    sl = slice(s * 16, (s + 1) * 16)
                    psum_t = scan_ps.tile([128, 6, 16], F32)
                    for m in (0, 1, 2, 3, 4, 5):
                        nc.tensor.matmul(psum_t[:, m, :],
                                         u0[:, m * 128:(m + 1) * 128], prev[:, 0, :],
                                         start=True, stop=False)
                        nc.tensor.matmul(psum_t[:, m, :],
                                         u1[:, m * 128:(m + 1) * 128], prev[:, 1, :],
                                         start=False, stop=True)
                    s_zr = gates.tile([128, 4, 16], F32)
                    nc.vector.tensor_add(s_zr, xp8[:, 0:4, sl], psum_t[:, 0:4, :])
                    zr = gates.tile([128, 4, 16], F32)
                    nc.scalar.activation(zr, s_zr, mybir.ActivationFunctionType.Sigmoid)
                    ph = gates.tile([128, 2, 16], F32)
                    nc.vector.tensor_add(ph, psum_t[:, 4:6, :], bh_rh)
                    t1 = gates.tile([128, 2, 16], F32)
                    nc.vector.tensor_mul(t1, zr[:, 2:4, :], ph)
                    t2 = gates.tile([128, 2, 16], F32)
                    nc.vector.tensor_add(t2, t1, xp8[:, 4:6, sl])
                    hh = gates.tile([128, 2, 16], F32)
                    nc.scalar.activation(hh, t2, mybir.ActivationFunctionType.Tanh)
                    dd = gates.tile([128, 2, 16], F32)
                    nc.vector.tensor_sub(dd, prev, hh)
                    ee = gates.tile([128, 2, 16], F32)
                    nc.vector.tensor_mul(ee, zr[:, 0:2, :], dd)
                    nc.vector.tensor_add(hist[:, :, s, :], hh, ee)
                    prev = hist[:, :, s, :]

                # ---- flush outputs: transpose back to batch-major ----------
                out_ap = out_h[:]
                for c in range(2):
                    ops_ = out_ps.tile([128, 128], F32, tag="ops")
                    nc.tensor.transpose(ops_, hist[:, c], identity)
                    osb = xts.tile([128, 128], F32, tag="osb")
                    nc.any.tensor_copy(osb, ops_)
                    # dst iteration order (s, b, f)
                    dst = bass.AP(
                        tensor=out_ap.tensor,
                        offset=out_ap.offset + g * NB * H + c * 128,
                        ap=[[H, NB], [t_total * H, BB], [1, 128]])
                    nc.sync.dma_start(out=dst, in_=osb)

            # ---- final state ----------------------------------------------
            st_ps = out_ps.tile([BB, 2, 128], F32, tag="ops")
            for c in range(2):
                nc.tensor.transpose(st_ps[:, c, :], prev[:, c, :], identity)
            st_sb = consts.tile([BB, 2, 128], F32)
            nc.any.tensor_copy(st_sb, st_ps)
            nc.sync.dma_start(out=st_h[:].rearrange("b (c f) -> b c f", c=2),
                              in_=st_sb)

    nc.compile()
    return nc


def kernel(x, hidden, W, U, b):
    key = x.shape[1]
    if key not in _cache:
        _cache[key] = _build(key)
    nc = _cache[key]
    in_maps = []
    for i in range(NCORES):
        sl = slice(i * BB, (i + 1) * BB)
        in_maps.append({
            "x": np.ascontiguousarray(x[sl], dtype=np.float32),
            "hidden": np.ascontiguousarray(hidden[sl], dtype=np.float32),
            "W": np.ascontiguousarray(W, dtype=np.float32),
            "U": np.ascontiguousarray(U, dtype=np.float32),
            "b": np.ascontiguousarray(b, dtype=np.float32),
        })
    res = run_bass_kernel_spmd(nc, in_maps, list(range(NCORES)))
    global LAST_RES
    LAST_RES = res
    out = np.concatenate([res.results[i]["out"] for i in range(NCORES)], axis=0)
    state = np.concatenate([res.results[i]["state"] for i in range(NCORES)], axis=0)
    return out, state


LAST_RES = None


# revision 14
# speedup vs baseline: 3.0003x; 3.0003x over previous
"""Trainium2 Bass kernel for a Keras-style GRU encoder (reset_after=True).

Math (per core, batch shard bb=16 of B=128):
  x_proj = x @ W + b_i          (precomputed for all T in a streaming GEMM)
  per t: rec = h @ U + b_r
         z = sigmoid(xp_z + rec_z); r = sigmoid(xp_r + rec_r)
         hh = tanh(xp_h + r * rec_h)
         h  = z*h + (1-z)*hh

Layouts: the scan runs feature-major ("transposed"): h^T is [128 part, 2, 16]
(feature chunk k on partitions, batch on free), rec^T accumulates in PSUM as
[128, 6, 16] via 12 matmuls with U chunks as the stationary operand. Gate
elementwise ops are [128, 2|4, 16] tiles (all 128 DVE/ACT lanes active).
x_proj is produced feature-major into a DRAM scratch ([6, 128, T*16],
col = t*16 + batch) by a streaming GEMM that PE-transposes x tiles; the
combined bias (b_i + b_r for z/r, b_i for the h gate) is folded in there.
Outputs transpose back to batch-major via PE every 8 steps.
"""
import sys

sys.path.insert(0, "/opt/trn_rl_repo")

import numpy as np
import concourse.bass as bass
import concourse.bacc as bacc
import concourse.tile as tile
from concourse import mybir
from concourse.bass_utils import run_bass_kernel_spmd
from concourse.masks import make_identity

B, T, IN, H = 128, 1024, 128, 256
G3 = 3 * H          # 768 = 6 chunks of 128
NCORES = 8
BB = B // NCORES    # 16 batch per core
NB = 8              # timesteps per output/fetch block
NG = T // NB
F32 = mybir.dt.float32
BF16 = mybir.dt.float16  # fp16: fast weight path, 4x finer mantissa than bf16

_cache = {}


def _build(t_total=T):
    ng = t_total // NB
    nc = bacc.Bacc("TRN2", target_bir_lowering=False, debug=False,
                   num_devices=NCORES)
    x_h = nc.declare_dram_parameter("x", [BB, t_total, IN], F32, isOutput=False)
    hid_h = nc.declare_dram_parameter("hidden", [BB, H], F32, isOutput=False)
    w_h = nc.declare_dram_parameter("W", [IN, G3], F32, isOutput=False)
    u_h = nc.declare_dram_parameter("U", [H, G3], F32, isOutput=False)
    b_h = nc.declare_dram_parameter("b", [2, G3], F32, isOutput=False)
    out_h = nc.declare_dram_parameter("out", [BB, t_total, H], F32, isOutput=True)
    st_h = nc.declare_dram_parameter("state", [BB, H], F32, isOutput=True)

    with tile.TileContext(nc) as tc:
        with (
            tc.tile_pool(name="consts", bufs=1) as consts,
            tc.tile_pool(name="xin", bufs=3) as xin,
            tc.tile_pool(name="xts", bufs=3) as xts,
            tc.tile_pool(name="xpsb", bufs=3) as xpsb,
            tc.tile_pool(name="xp8", bufs=3) as xp8p,
            tc.tile_pool(name="hist", bufs=3) as histp,
            tc.tile_pool(name="gates", bufs=3) as gates,
            tc.tile_pool(name="gemm_ps", bufs=1, space="PSUM") as gemm_ps,
            tc.tile_pool(name="scan_ps", bufs=2, space="PSUM") as scan_ps,
            tc.tile_pool(name="out_ps", bufs=2, space="PSUM") as out_ps,
            tc.tile_pool(name="dram", bufs=1, space="DRAM") as dram,
        ):
            # ---- constants -------------------------------------------------
            identity = consts.tile([128, 128], F32)
            make_identity(nc, identity)
            # U and W live as bf16 stationary operands (fp32 weight loads are
            # ~4x slower on the PE weight path); PSUM still accumulates fp32.
            u0f = consts.tile([128, G3], F32)
            u1f = consts.tile([128, G3], F32)
            nc.sync.dma_start(out=u0f, in_=u_h[0:128, :])
            nc.sync.dma_start(out=u1f, in_=u_h[128:256, :])
            u0 = consts.tile([128, G3], BF16)
            u1 = consts.tile([128, G3], BF16)
            nc.vector.tensor_copy(u0, u0f)
            nc.vector.tensor_copy(u1, u1f)
            w_f = consts.tile([128, G3], F32)
            nc.sync.dma_start(out=w_f, in_=w_h[:])
            w_sb = consts.tile([128, G3], BF16)
            nc.vector.tensor_copy(w_sb, w_f)
            # b [2, 768] -> [128 part(p), 2, 6(m)]
            b_sb = consts.tile([128, 2, 6], F32)
            nc.sync.dma_start(out=b_sb,
                              in_=b_h[:].rearrange("two (m p) -> p two m", p=128))
            # per-(partition, chunk) bias column folded into x_proj:
            # z/r chunks get b_i + b_r; h chunks get b_i only.
            bias_cols = consts.tile([128, 6], F32)
            nc.vector.tensor_add(bias_cols, b_sb[:, 0, :], b_sb[:, 1, :])
            nc.vector.tensor_copy(bias_cols[:, 4:6], b_sb[:, 0, 4:6])
            # recurrent bias for the h gate, broadcast over batch: [128, 2, 16]
            zeros16 = consts.tile([128, 16], F32)
            nc.vector.memset(zeros16, 0.0)
            bh_rh = consts.tile([128, 2, 16], F32)
            for c in range(2):
                nc.vector.tensor_scalar_add(bh_rh[:, c, :], zeros16,
                                            b_sb[:, 1, 4 + c:5 + c])

            # ---- initial hidden -> hT_init [128, 2, 16] --------------------
            hid_sb = consts.tile([BB, H], F32)
            nc.sync.dma_start(out=hid_sb, in_=hid_h[:])
            hid_ps = out_ps.tile([128, 2, 16], F32, tag="ops")
            for c in range(2):
                nc.tensor.transpose(hid_ps[:, c, :], hid_sb[:, c * 128:(c + 1) * 128],
                                    identity[0:BB, 0:BB])
            ht_init = consts.tile([128, 2, 16], F32)
            nc.vector.tensor_copy(ht_init, hid_ps)
            ht_init_b = consts.tile([128, 2, 16], BF16)
            nc.vector.tensor_copy(ht_init_b, hid_ps)

            # x_proj^T scratch: [6(m), 128(p), t*16 + j]
            xpt_d = dram.tile([6, 128, t_total * BB], F32)

            prev = ht_init      # f32 view of h_{t-1} (gate math)
            prev_b = ht_init_b  # bf16 copy (matmul rhs)
            out_ap = out_h[:]
            x_ap = x_h[:]

            # Background (non-chain) work is emitted as small pieces popped
            # between scan steps, so in-order engine queues never insert a
            # long burst of GEMM/flush work into the recurrence chain.
            work_q = []

            def gemm_pieces(g):
                st = {}

                def load(g=g):
                    x_tile = xin.tile([128, IN], F32)
                    src = bass.AP(tensor=x_ap.tensor,
                                  offset=x_ap.offset + g * NB * IN,
                                  ap=[[IN, NB], [t_total * IN, BB], [1, IN]])
                    nc.sync.dma_start(out=x_tile, in_=src)
                    xt_ps = out_ps.tile([128, 128], F32, tag="ops")
                    nc.tensor.transpose(xt_ps, x_tile, identity)
                    xt_sb = xts.tile([128, 128], BF16)
                    nc.any.tensor_copy(xt_sb, xt_ps)
                    st["xt"] = xt_sb
                    st["ps"] = gemm_ps.tile([128, 6, 128], F32, name="gps", tag="gps")
                    st["xp"] = xpsb.tile([128, 6, 128], F32, name="xpg", tag="xp")

                def mm(m):
                    def fn():
                        nc.tensor.matmul(st["ps"][:, m, :],
                                         w_sb[:, m * 128:(m + 1) * 128],
                                         st["xt"], start=True, stop=True)
                        nc.any.tensor_scalar_add(st["xp"][:, m, :],
                                                 st["ps"][:, m, :],
                                                 bias_cols[:, m:m + 1])
                    return fn

                def store(g=g):
                    nc.sync.dma_start(
                        out=xpt_d[:, :, g * 128:(g + 1) * 128]
                            .rearrange("m p c -> p m c"),
                        in_=st["xp"])

                return [load] + [mm(m) for m in range(6)] + [store]

            def flush_pieces(g, hist):
                def piece(c, g=g, hist=hist):
                    def fn():
                        ops_ = out_ps.tile([128, 128], F32, tag="ops")
                        nc.tensor.transpose(ops_, hist[:, c], identity)
                        osb = xts.tile([128, 128], F32, tag="osb")
                        nc.any.tensor_copy(osb, ops_)
                        dst = bass.AP(
                            tensor=out_ap.tensor,
                            offset=out_ap.offset + g * NB * H + c * 128,
                            ap=[[H, NB], [t_total * H, BB], [1, 128]])
                        nc.sync.dma_start(out=dst, in_=osb)
                    return fn
                return [piece(0), piece(1)]

            # prologue: first two GEMM blocks emitted eagerly
            for fn in gemm_pieces(0) + (gemm_pieces(1) if ng > 1 else []):
                fn()

            prev_hist = None
            for g in range(ng):
                while work_q:           # drain leftovers before the xp8 read
                    work_q.pop(0)()
                xp8 = xp8p.tile([128, 6, NB * BB], F32)
                nc.sync.dma_start(
                    out=xp8,
                    in_=xpt_d[:, :, g * 128:(g + 1) * 128]
                        .rearrange("m p c -> p m c"))
                if prev_hist is not None:
                    work_q.extend(flush_pieces(g - 1, prev_hist))
                if g + 2 < ng:
                    work_q.extend(gemm_pieces(g + 2))
                hist = histp.tile([128, 2, NB, 16], F32)
                for s in range(NB):
                    sl = slice(s * 16, (s + 1) * 16)
                    psum_zr = scan_ps.tile([128, 4, 16], F32)
                    psum_h = scan_ps.tile([128, 2, 16], F32)
                    for m in (0, 1, 2, 3):
                        nc.tensor.matmul(psum_zr[:, m, :],
                                         u0[:, m * 128:(m + 1) * 128],
                                         prev_b[:, 0, :], start=True, stop=False)
                        nc.tensor.matmul(psum_zr[:, m, :],
                                         u1[:, m * 128:(m + 1) * 128],
                                         prev_b[:, 1, :], start=False, stop=True)
                    for m in (4, 5):
                        nc.tensor.matmul(psum_h[:, m - 4, :],
                                         u0[:, m * 128:(m + 1) * 128],
                                         prev_b[:, 0, :], start=True, stop=False)
                        nc.tensor.matmul(psum_h[:, m - 4, :],
                                         u1[:, m * 128:(m + 1) * 128],
                                         prev_b[:, 1, :], start=False, stop=True)
                    s_zr = gates.tile([128, 4, 16], F32)
                    nc.vector.tensor_add(s_zr, xp8[:, 0:4, sl], psum_zr)
                    zr = gates.tile([128, 4, 16], F32)
                    nc.scalar.activation(zr, s_zr, mybir.ActivationFunctionType.Sigmoid)
                    ph = gates.tile([128, 2, 16], F32)
                    nc.vector.tensor_add(ph, psum_h, bh_rh)
                    t1 = gates.tile([128, 2, 16], F32)
                    nc.vector.tensor_mul(t1, zr[:, 2:4, :], ph)
                    t2 = gates.tile([128, 2, 16], F32)
                    nc.vector.tensor_add(t2, t1, xp8[:, 4:6, sl])
                    hh = gates.tile([128, 2, 16], F32)
                    nc.scalar.activation(hh, t2, mybir.ActivationFunctionType.Tanh)
                    dd = gates.tile([128, 2, 16], F32)
                    nc.vector.tensor_sub(dd, prev, hh)
                    ee = gates.tile([128, 2, 16], F32)
                    nc.vector.tensor_mul(ee, zr[:, 0:2, :], dd)
                    nc.vector.tensor_add(hist[:, :, s, :], hh, ee)
                    prev = hist[:, :, s, :]
                    hb = gates.tile([128, 2, 16], BF16)
                    nc.vector.tensor_copy(hb, prev)
                    prev_b = hb
                    # sprinkle background work between steps
                    for _ in range(2):
                        if work_q:
                            work_q.pop(0)()
                prev_hist = hist
            while work_q:
                work_q.pop(0)()
            for fn in flush_pieces(ng - 1, prev_hist):
                fn()

            # ---- final state ----------------------------------------------
            st_ps = out_ps.tile([BB, 2, 128], F32, tag="ops")
            for c in range(2):
                nc.tensor.transpose(st_ps[:, c, :], prev[:, c, :], identity)
            st_sb = consts.tile([BB, 2, 128], F32)
            nc.any.tensor_copy(st_sb, st_ps)
            nc.sync.dma_start(out=st_h[:].rearrange("b (c f) -> b c f", c=2),
                              in_=st_sb)

    nc.compile()
    return nc


def kernel(x, hidden, W, U, b):
    key = x.shape[1]
    if key not in _cache:
        _cache[key] = _build(key)
    nc = _cache[key]
    in_maps = []
    for i in range(NCORES):
        sl = slice(i * BB, (i + 1) * BB)
        in_maps.append({
            "x": np.ascontiguousarray(x[sl], dtype=np.float32),
            "hidden": np.ascontiguousarray(hidden[sl], dtype=np.float32),
            "W": np.ascontiguousarray(W, dtype=np.float32),
            "U": np.ascontiguousarray(U, dtype=np.float32),
            "b": np.ascontiguousarray(b, dtype=np.float32),
        })
    res = run_bass_kernel_spmd(nc, in_maps, list(range(NCORES)))
    global LAST_RES
    LAST_RES = res
    out = np.concatenate([res.results[i]["out"] for i in range(NCORES)], axis=0)
    state = np.concatenate([res.results[i]["state"] for i in range(NCORES)], axis=0)
    return out, state


LAST_RES = None


# revision 24
# speedup vs baseline: 3.7062x; 1.2353x over previous
"""Trainium2 Bass kernel for a Keras-style GRU encoder (reset_after=True).

Math (per core, batch shard bb=16 of B=128):
  x_proj = x @ W + b_i          (precomputed for all T in a streaming GEMM)
  per t: rec = h @ U + b_r
         z = sigmoid(xp_z + rec_z); r = sigmoid(xp_r + rec_r)
         hh = tanh(xp_h + r * rec_h)
         h  = z*h + (1-z)*hh

Layouts: the scan runs feature-major ("transposed"): h^T is [128 part, 2, 16]
(feature chunk k on partitions, batch on free), rec^T accumulates in PSUM as
[128, 6, 16] via 12 matmuls with U chunks as the stationary operand. Gate
elementwise ops are [128, 2|4, 16] tiles (all 128 DVE/ACT lanes active).
x_proj is produced feature-major into a DRAM scratch ([6, 128, T*16],
col = t*16 + batch) by a streaming GEMM that PE-transposes x tiles; the
combined bias (b_i + b_r for z/r, b_i for the h gate) is folded in there.
Outputs transpose back to batch-major via PE every 8 steps.
"""
import sys

sys.path.insert(0, "/opt/trn_rl_repo")

import numpy as np
import concourse.bass as bass
import concourse.bacc as bacc
import concourse.tile as tile
from concourse import mybir
from concourse.bass_utils import run_bass_kernel_spmd
from concourse.masks import make_identity

B, T, IN, H = 128, 1024, 128, 256
G3 = 3 * H          # 768 = 6 chunks of 128
NCORES = 8
BB = B // NCORES    # 16 batch per core
NB = 8              # timesteps per output/fetch block
NG = T // NB
F32 = mybir.dt.float32
BF16 = mybir.dt.float16  # fp16: fast weight path, 4x finer mantissa than bf16

_cache = {}


def _build(t_total=T):
    ng = t_total // NB
    nc = bacc.Bacc("TRN2", target_bir_lowering=False, debug=False,
                   num_devices=NCORES)
    x_h = nc.declare_dram_parameter("x", [BB, t_total, IN], F32, isOutput=False)
    hid_h = nc.declare_dram_parameter("hidden", [BB, H], F32, isOutput=False)
    w_h = nc.declare_dram_parameter("W", [IN, G3], F32, isOutput=False)
    u_h = nc.declare_dram_parameter("U", [H, G3], F32, isOutput=False)
    b_h = nc.declare_dram_parameter("b", [2, G3], F32, isOutput=False)
    out_h = nc.declare_dram_parameter("out", [BB, t_total, H], F32, isOutput=True)
    st_h = nc.declare_dram_parameter("state", [BB, H], F32, isOutput=True)

    with tile.TileContext(nc) as tc:
        with (
            tc.tile_pool(name="consts", bufs=1) as consts,
            tc.tile_pool(name="xin", bufs=3) as xin,
            tc.tile_pool(name="xts", bufs=3) as xts,
            tc.tile_pool(name="xpsb", bufs=3) as xpsb,
            tc.tile_pool(name="xp8", bufs=3) as xp8p,
            tc.tile_pool(name="hist", bufs=3) as histp,
            tc.tile_pool(name="gates", bufs=3) as gates,
            tc.tile_pool(name="gemm_ps", bufs=1, space="PSUM") as gemm_ps,
            tc.tile_pool(name="scan_ps", bufs=2, space="PSUM") as scan_ps,
            tc.tile_pool(name="out_ps", bufs=2, space="PSUM") as out_ps,
            tc.tile_pool(name="dram", bufs=1, space="DRAM") as dram,
        ):
            # ---- constants -------------------------------------------------
            identity = consts.tile([128, 128], F32)
            make_identity(nc, identity)
            # U and W live as bf16 stationary operands (fp32 weight loads are
            # ~4x slower on the PE weight path); PSUM still accumulates fp32.
            u0f = consts.tile([128, G3], F32)
            u1f = consts.tile([128, G3], F32)
            nc.sync.dma_start(out=u0f, in_=u_h[0:128, :])
            nc.sync.dma_start(out=u1f, in_=u_h[128:256, :])
            u0 = consts.tile([128, G3], BF16)
            u1 = consts.tile([128, G3], BF16)
            nc.vector.tensor_copy(u0, u0f)
            nc.vector.tensor_copy(u1, u1f)
            w_f = consts.tile([128, G3], F32)
            nc.sync.dma_start(out=w_f, in_=w_h[:])
            w_sb = consts.tile([128, G3], BF16)
            nc.vector.tensor_copy(w_sb, w_f)
            # b [2, 768] -> [128 part(p), 2, 6(m)]
            b_sb = consts.tile([128, 2, 6], F32)
            nc.sync.dma_start(out=b_sb,
                              in_=b_h[:].rearrange("two (m p) -> p two m", p=128))
            # per-(partition, chunk) bias column folded into x_proj:
            # z/r chunks get b_i + b_r; h chunks get b_i only.
            bias_cols = consts.tile([128, 6], F32)
            nc.vector.tensor_add(bias_cols, b_sb[:, 0, :], b_sb[:, 1, :])
            nc.vector.tensor_copy(bias_cols[:, 4:6], b_sb[:, 0, 4:6])
            # recurrent bias for the h gate, broadcast over batch: [128, 2, 16]
            zeros16 = consts.tile([128, 16], F32)
            nc.vector.memset(zeros16, 0.0)
            bh_rh = consts.tile([128, 2, 16], F32)
            for c in range(2):
                nc.vector.tensor_scalar_add(bh_rh[:, c, :], zeros16,
                                            b_sb[:, 1, 4 + c:5 + c])

            # fp16 identity: stationary operand for the xp->PSUM accumulate
            # matmul and for transposes of fp16 tiles.
            id16 = consts.tile([128, 128], BF16)
            nc.vector.tensor_copy(id16, identity)

            # ---- initial hidden -> hT_init [128, 2, 16] --------------------
            hid_sb = consts.tile([BB, H], F32)
            nc.sync.dma_start(out=hid_sb, in_=hid_h[:])
            hid_ps = out_ps.tile([128, 2, 16], F32, tag="ops")
            for c in range(2):
                nc.tensor.transpose(hid_ps[:, c, :], hid_sb[:, c * 128:(c + 1) * 128],
                                    identity[0:BB, 0:BB])
            ht_init = consts.tile([128, 2, 16], BF16)
            nc.vector.tensor_copy(ht_init, hid_ps)

            # x_proj^T scratch, step-major fp16: [128(p), t, 6(m), 16(j)]
            xpt_d = dram.tile([128, t_total, 6, BB], BF16)

            prev = ht_init      # fp16 [128, 2, 16] view of h_{t-1}
            out_ap = out_h[:]
            x_ap = x_h[:]

            # Background (non-chain) work is emitted as small pieces popped
            # between scan steps, so in-order engine queues never insert a
            # long burst of GEMM/flush work into the recurrence chain.
            work_q = []

            def gemm_pieces(g):
                st = {}

                def load(g=g):
                    x_tile = xin.tile([128, IN], F32)
                    src = bass.AP(tensor=x_ap.tensor,
                                  offset=x_ap.offset + g * NB * IN,
                                  ap=[[IN, NB], [t_total * IN, BB], [1, IN]])
                    nc.sync.dma_start(out=x_tile, in_=src)
                    xt_ps = out_ps.tile([128, 128], F32, tag="ops")
                    nc.tensor.transpose(xt_ps, x_tile, identity)
                    xt_sb = xts.tile([128, 128], BF16)
                    nc.any.tensor_copy(xt_sb, xt_ps)
                    st["xt"] = xt_sb
                    st["ps"] = gemm_ps.tile([128, 6, 128], F32, name="gps", tag="gps")
                    st["xp"] = xpsb.tile([128, NB, 6, BB], BF16, name="xpg", tag="xp")

                def mm(m):
                    def fn():
                        nc.tensor.matmul(st["ps"][:, m, :],
                                         w_sb[:, m * 128:(m + 1) * 128],
                                         st["xt"], start=True, stop=True)
                        # psum cols are (s, j); scatter into step-major xp
                        # layout [s, m, j], casting to fp16.
                        nc.any.tensor_scalar_add(
                            st["xp"][:, :, m, :],
                            st["ps"][:, m, :].rearrange("p (s j) -> p s j", s=NB),
                            bias_cols[:, m:m + 1])
                    return fn

                def store(g=g):
                    nc.sync.dma_start(
                        out=xpt_d[:, g * NB:(g + 1) * NB], in_=st["xp"])

                return [load] + [mm(m) for m in range(6)] + [store]

            def flush_pieces(g, hist):
                def piece(c, g=g, hist=hist):
                    def fn():
                        ops_ = out_ps.tile([128, 128], BF16, tag="ops")
                        nc.tensor.transpose(ops_, hist[:, c], id16)
                        osb = xts.tile([128, 128], F32, tag="osb")
                        nc.any.tensor_copy(osb, ops_)
                        dst = bass.AP(
                            tensor=out_ap.tensor,
                            offset=out_ap.offset + g * NB * H + c * 128,
                            ap=[[H, NB], [t_total * H, BB], [1, 128]])
                        nc.sync.dma_start(out=dst, in_=osb)
                    return fn
                return [piece(0), piece(1)]

            # prologue: first two GEMM blocks emitted eagerly
            for fn in gemm_pieces(0) + (gemm_pieces(1) if ng > 1 else []):
                fn()

            prev_hist = None
            for g in range(ng):
                while work_q:           # drain leftovers before the xp8 read
                    work_q.pop(0)()
                xp8 = xp8p.tile([128, NB, 6, BB], BF16)
                nc.sync.dma_start(out=xp8, in_=xpt_d[:, g * NB:(g + 1) * NB])
                if prev_hist is not None:
                    work_q.extend(flush_pieces(g - 1, prev_hist))
                if g + 2 < ng:
                    work_q.extend(gemm_pieces(g + 2))
                hist = histp.tile([128, 2, NB, 16], BF16)
                for s in range(NB):
                    psum_zr = scan_ps.tile([128, 4, 16], F32)
                    psum_h = scan_ps.tile([128, 2, 16], F32)
                    # xp_zr lands in PSUM first via identity weights (one
                    # N=64 matmul), then the U matmuls accumulate on top: the
                    # sigmoid reads (x_proj + rec) straight from PSUM.
                    nc.tensor.matmul(psum_zr.rearrange("p a b -> p (a b)"), id16,
                                     xp8[:, s, 0:4, :].rearrange("p a b -> p (a b)"),
                                     start=True, stop=False,
                                     skip_group_check=True)
                    for m in (0, 1, 2, 3):
                        nc.tensor.matmul(psum_zr[:, m, :],
                                         u0[:, m * 128:(m + 1) * 128],
                                         prev[:, 0, :], start=False, stop=False,
                                         skip_group_check=True)
                        nc.tensor.matmul(psum_zr[:, m, :],
                                         u1[:, m * 128:(m + 1) * 128],
                                         prev[:, 1, :], start=False, stop=(m == 3),
                                         skip_group_check=True)
                    for m in (4, 5):
                        nc.tensor.matmul(psum_h[:, m - 4, :],
                                         u0[:, m * 128:(m + 1) * 128],
                                         prev[:, 0, :], start=True, stop=False)
                        nc.tensor.matmul(psum_h[:, m - 4, :],
                                         u1[:, m * 128:(m + 1) * 128],
                                         prev[:, 1, :], start=False, stop=True)
                    zr = gates.tile([128, 4, 16], F32)
                    nc.scalar.activation(zr, psum_zr,
                                         mybir.ActivationFunctionType.Sigmoid)
                    ph = gates.tile([128, 2, 16], F32)
                    nc.vector.tensor_add(ph, psum_h, bh_rh)
                    # z-only products, off the DVE chain (gpsimd):
                    omz = gates.tile([128, 2, 16], F32)
                    nc.gpsimd.tensor_scalar(omz, zr[:, 0:2, :], -1.0, 1.0,
                                            mybir.AluOpType.mult,
                                            mybir.AluOpType.add)
                    qq = gates.tile([128, 2, 16], F32)
                    nc.gpsimd.tensor_mul(qq, zr[:, 0:2, :], prev)
                    t1 = gates.tile([128, 2, 16], F32)
                    nc.vector.tensor_mul(t1, zr[:, 2:4, :], ph)
                    t2 = gates.tile([128, 2, 16], F32)
                    nc.vector.tensor_add(t2, t1, xp8[:, s, 4:6, :])
                    hh = gates.tile([128, 2, 16], F32)
                    nc.scalar.activation(hh, t2, mybir.ActivationFunctionType.Tanh)
                    ff = gates.tile([128, 2, 16], F32)
                    nc.vector.tensor_mul(ff, omz, hh)
                    nc.vector.tensor_add(hist[:, :, s, :], ff, qq)
                    prev = hist[:, :, s, :]
                    # sprinkle background work between steps
                    for _ in range(2):
                        if work_q:
                            work_q.pop(0)()
                prev_hist = hist
            while work_q:
                work_q.pop(0)()
            for fn in flush_pieces(ng - 1, prev_hist):
                fn()

            # ---- final state ----------------------------------------------
            st_ps = out_ps.tile([BB, 2, 128], BF16, tag="ops")
            for c in range(2):
                nc.tensor.transpose(st_ps[:, c, :], prev[:, c, :], id16)
            st_sb = consts.tile([BB, 2, 128], F32)
            nc.any.tensor_copy(st_sb, st_ps)
            nc.sync.dma_start(out=st_h[:].rearrange("b (c f) -> b c f", c=2),
                              in_=st_sb)

    nc.compile()
    return nc


def kernel(x, hidden, W, U, b):
    key = x.shape[1]
    if key not in _cache:
        _cache[key] = _build(key)
    nc = _cache[key]
    in_maps = []
    for i in range(NCORES):
        sl = slice(i * BB, (i + 1) * BB)
        in_maps.append({
            "x": np.ascontiguousarray(x[sl], dtype=np.float32),
            "hidden": np.ascontiguousarray(hidden[sl], dtype=np.float32),
            "W": np.ascontiguousarray(W, dtype=np.float32),
            "U": np.ascontiguousarray(U, dtype=np.float32),
            "b": np.ascontiguousarray(b, dtype=np.float32),
        })
    import time as _time
    res = None
    for attempt in range(3):
        try:
            res = run_bass_kernel_spmd(nc, in_maps, list(range(NCORES)))
            break
        except Exception:
            if attempt == 2:
                raise
            _time.sleep(15)
    global LAST_RES
    LAST_RES = res
    out = np.concatenate([res.results[i]["out"] for i in range(NCORES)], axis=0)
    state = np.concatenate([res.results[i]["state"] for i in range(NCORES)], axis=0)
    return out, state


LAST_RES = None
